# revision 8
# baseline (speedup 1.0000x reference)
"""Trainium2 Bass kernel for nn_CrossAttention (B=4, C=256, N=64*64=4096, CQK=32).

Reference computation:
    q = Wq @ xf + bq          [B, N, 32]
    k = Wk @ yf + bk          [B, 32, N]
    v = Wv @ yf + bv          [B, 256, N]
    attn = softmax(q @ k)     [B, N, N]
    out = gamma * (v @ attn^T) + x

Sharding: 8 cores = batch(4) x query-half(2). Each core owns 2048 query
positions of one sample and all 4096 keys of that sample.

Per-core design (v2 - compound-matmul restructure):
  - qT [32, n] / kT [32, m] bf16, zero-padded to 128 partitions (fast weight
    load), produced by compound projection matmuls.
  - energy eT[m-chunk, n] via ONE compound matmul per key chunk: kT chunk is
    the stationary operand, all n=2048 queries stream, output spans 4 PSUM
    banks. 1 LDWEIGHTS per 2048 output columns (the baseline paid 4).
  - exp on the scalar engine at full 2048 width (32 activations instead of
    128: the 352-cycle fixed cost per ACTIVATE amortizes 4x). Output is the
    fp8 DoubleRow pair tile ex[t][:, r, :] = exp(energy of chunk 2t+r).
  - AV with v as the STATIONARY side: out[e-chunk, n] accumulates over the 16
    key pairs; each pair contributes one compound fp8-DoubleRow matmul that
    streams the exp tile. Output arrives directly in [e, n] orientation - the
    final layout - so the baseline's 32 PE transposes disappear entirely.
  - softmax denominator: an all-ones(*1/gamma) fp8 stationary operand makes
    the PE compute dn[p, n] = denom[n]/gamma broadcast across partitions;
    scalar-engine Reciprocal (idle after the exps) turns it into
    gamma/denom[n], so normalize+scale is one tensor_tensor multiply.
  - residual: x arrives fp32 late, gets gamma*bv folded in (xg = x + g*bv),
    final out = av * recipb + xg, two vector ops per tile, DMA out fp32.
"""

import contextlib

import numpy as np

import concourse.mybir as mybir
import concourse.tile as tile
from concourse import bacc
from concourse.bass_utils import run_bass_kernel_spmd

F32 = mybir.dt.float32
F8 = mybir.dt.float8e4
BF16 = mybir.dt.bfloat16
AFT = mybir.ActivationFunctionType
DR = mybir.MatmulPerfMode.DoubleRow

B = 4
C = 256
CQK = 32
N = 4096  # 64 * 64
NCORES = 8
NLOC = N // 2  # 2048 queries per core
CCH = C // 128  # 2 channel chunks
MC = N // 128  # 32 key chunks
NP = MC // 2  # 16 key pairs (DoubleRow)
HALF = NLOC // 2  # 1024: AV processes queries in halves (PSUM budget)


def _trace_kernel(
    ctx, tc, x_d, xb_d, y_d, wq_d, wk_d, wv_d, bq_d, bk_d, bv_d, g_d, out_d
):
    nc = tc.nc

    const = ctx.enter_context(tc.tile_pool(name="const", bufs=1))
    big = ctx.enter_context(tc.tile_pool(name="big", bufs=1))
    vaugp = ctx.enter_context(tc.tile_pool(name="vaugp", bufs=NP))
    expp = ctx.enter_context(tc.tile_pool(name="expp", bufs=NP))
    recp = ctx.enter_context(tc.tile_pool(name="recp", bufs=2))
    finp = ctx.enter_context(tc.tile_pool(name="finp", bufs=2))

    # ---- constant / weight loads (weights pre-cast to bf16 on host) ----
    wq_b = const.tile([128, CCH, CQK], BF16, tag="wq_b")
    nc.sync.dma_start(out=wq_b, in_=wq_d.ap())
    wk_b = const.tile([128, CCH, CQK], BF16, tag="wk_b")
    nc.sync.dma_start(out=wk_b, in_=wk_d.ap())
    wv_b = const.tile([128, CCH, C], BF16, tag="wv_b")
    nc.sync.dma_start(out=wv_b, in_=wv_d.ap())
    bq_sb = const.tile([CQK, 1], F32, tag="bq_sb")
    nc.sync.dma_start(out=bq_sb, in_=bq_d.ap())
    bk_sb = const.tile([CQK, 1], F32, tag="bk_sb")
    nc.sync.dma_start(out=bk_sb, in_=bk_d.ap())
    bv_sb = const.tile([128, CCH], F32, tag="bv_sb")
    nc.sync.dma_start(out=bv_sb, in_=bv_d.ap())
    g_sb = const.tile([128, 1], F32, tag="g_sb")
    nc.sync.dma_start(out=g_sb, in_=g_d.ap())
    gbv_sb = const.tile([128, CCH], F32, tag="gbv_sb")
    nc.vector.tensor_scalar_mul(gbv_sb, bv_sb, g_sb)
    rg_sb = const.tile([128, 1], F32, tag="rg_sb")
    nc.vector.reciprocal(rg_sb, g_sb)
    # all-(1/gamma) stationary operand for the denominator matmuls
    ones_g = const.tile([128, 2, 128], F8, tag="ones_g")
    nc.vector.memset(ones_g, 1.0)
    nc.vector.tensor_scalar_mul(ones_g, ones_g, rg_sb)

    # ---- activations in: y and x_b arrive bf16 from host (critical path);
    # fp32 x (residual only) is DMA'd last so it overlaps the exp phase.
    # y on the gpsimd ring, x_b on the vector ring (scalar stays DMA-free:
    # it is the bottleneck engine during the exp phase).
    NDMA = 8
    x_b = []
    for cc in range(CCH):
        x_bt = big.tile([128, NLOC], BF16, tag=f"x_b{cc}", name=f"x_b{cc}")
        nc.sync.dma_start(out=x_bt[:, : NLOC // 2], in_=xb_d.ap()[cc, :, : NLOC // 2])
        nc.sync.dma_start(out=x_bt[:, NLOC // 2 :], in_=xb_d.ap()[cc, :, NLOC // 2 :])
        x_b.append(x_bt)
    y_b = [
        big.tile([128, N], BF16, tag=f"y_b{cc}", name=f"y_b{cc}")
        for cc in range(CCH)
    ]
    for d in range(NDMA):
        sl = slice(d * (N // NDMA), (d + 1) * (N // NDMA))
        for cc in range(CCH):
            nc.gpsimd.dma_start(out=y_b[cc][:, sl], in_=y_d.ap()[cc, :, sl])
    xg = []
    for cc in range(CCH):
        x_t = big.tile([128, NLOC], F32, tag=f"xg{cc}", name=f"xg{cc}")
        xg.append(x_t)

    # ---- q/k projections: compound matmuls, bias-add + cast to bf16 ----
    kT_sb = big.tile([128, N], BF16, tag="kT_sb")
    nc.gpsimd.memset(kT_sb, 0.0)
    qT_sb = big.tile([128, NLOC], BF16, tag="qT_sb")
    nc.gpsimd.memset(qT_sb, 0.0)
    with contextlib.ExitStack() as pctx:
        ppq = pctx.enter_context(tc.tile_pool(name="ppq", bufs=2, space="PSUM"))
        for nt in range(2):  # key halves of 2048
            msl = slice(nt * 2048, (nt + 1) * 2048)
            pk = ppq.tile([CQK, 2048], F32, tag="pp", name=f"pk{nt}")
            for s in range(4):
                ssl = slice(s * 512, (s + 1) * 512)
                gsl = slice(nt * 2048 + s * 512, nt * 2048 + (s + 1) * 512)
                for cc in range(CCH):
                    nc.tensor.matmul(
                        pk[:, ssl],
                        lhsT=wk_b[:, cc, :],
                        rhs=y_b[cc][:, gsl],
                        start=(cc == 0),
                        stop=(cc == CCH - 1),
                    )
            nc.vector.tensor_scalar_add(kT_sb[0:CQK, msl], pk, bk_sb)
        pq = ppq.tile([CQK, NLOC], F32, tag="pp", name="pq")
        for s in range(4):
            ssl = slice(s * 512, (s + 1) * 512)
            for cc in range(CCH):
                nc.tensor.matmul(
                    pq[:, ssl],
                    lhsT=wq_b[:, cc, :],
                    rhs=x_b[cc][:, ssl],
                    start=(cc == 0),
                    stop=(cc == CCH - 1),
                )
        nc.vector.tensor_scalar_add(qT_sb[0:CQK, :], pq, bq_sb)

    # ---- v projection -> fp8 DoubleRow pair tiles vaug[t][p, r, e] ----
    # vaug[t][p, r, e] = v[m = 256*t + 128*r + p, e]
    vaug = []
    with contextlib.ExitStack() as pctx:
        pvp = pctx.enter_context(tc.tile_pool(name="pvp", bufs=2, space="PSUM"))
        for t in range(NP):
            va = vaugp.tile([128, 2, C], F8, tag="vaug", name=f"vaug{t}")
            for r in range(2):
                mc = 2 * t + r
                pv = pvp.tile([128, C], F32, tag="pv", name=f"pv{mc}")
                for cc in range(CCH):
                    nc.tensor.matmul(
                        pv,
                        lhsT=y_b[cc][:, mc * 128 : (mc + 1) * 128],
                        rhs=wv_b[:, cc, :],
                        start=(cc == 0),
                        stop=(cc == CCH - 1),
                    )
                nc.vector.tensor_copy(va[:, r, :], pv)
            vaug.append(va)

    # fp32 x for the residual: streams in during the exp phase; fold g*bv in.
    for cc in range(CCH):
        for d in range(2):
            sl = slice(d * (NLOC // 2), (d + 1) * (NLOC // 2))
            nc.sync.dma_start(out=xg[cc][:, sl], in_=x_d.ap()[cc, :, sl])
        nc.vector.tensor_scalar_add(xg[cc], xg[cc], gbv_sb[:, cc : cc + 1])

    # ---- energy + exp: one compound matmul + one 2048-wide ACTIVATE per
    # key chunk; 4-bank PSUM tiles double-buffered ----
    ex = [
        expp.tile([128, 2, NLOC], F8, tag="exp", name=f"ex{t}") for t in range(NP)
    ]
    scr = const.tile([128, NLOC], BF16, tag="scr")  # 2x-activation probe
    with contextlib.ExitStack() as pctx:
        pep = pctx.enter_context(tc.tile_pool(name="pep", bufs=2, space="PSUM"))
        for mc in range(MC):
            t, r = divmod(mc, 2)
            pe_t = pep.tile([128, NLOC], F32, tag="pe", name=f"pe{mc}")
            for s in range(4):
                ssl = slice(s * 512, (s + 1) * 512)
                nc.tensor.matmul(
                    pe_t[:, ssl],
                    lhsT=kT_sb[:, mc * 128 : (mc + 1) * 128],
                    rhs=qT_sb[:, ssl],
                    start=True,
                    stop=True,
                )
            if mc == 0:
                # probe: does a bf16-out ACTIVATE run at 2x? (trace check)
                nc.scalar.activation(scr, pe_t, AFT.Exp)
                nc.vector.tensor_copy(ex[t][:, r, :], scr)
            else:
                nc.scalar.activation(ex[t][:, r, :], pe_t, AFT.Exp)

    # ---- AV + denominator + normalize, per query half ----
    with contextlib.ExitStack() as pctx:
        dnp = pctx.enter_context(tc.tile_pool(name="dnp", bufs=1, space="PSUM"))
        avp = pctx.enter_context(tc.tile_pool(name="avp", bufs=3, space="PSUM"))
        for h in range(2):
            hsl = slice(h * HALF, (h + 1) * HALF)
            dn = dnp.tile([128, HALF], F32, tag="dn", name=f"dn{h}")
            for t in range(NP):
                for s in range(2):
                    ssl = slice(s * 512, (s + 1) * 512)
                    gsl = slice(h * HALF + s * 512, h * HALF + (s + 1) * 512)
                    nc.tensor.matmul(
                        dn[:, ssl],
                        lhsT=ones_g,
                        rhs=ex[t][:, :, gsl],
                        start=(t == 0),
                        stop=(t == NP - 1),
                        perf_mode=DR,
                    )
            # recipb = gamma / denom (dn = denom/gamma) via exp(-ln(x)),
            # broadcast on 128 partitions; scalar engine is idle here and
            # Ln/Exp share one activation table set
            lnt = recp.tile([128, HALF], F32, tag="lnt", name=f"lnt{h}")
            nc.scalar.activation(lnt, dn, AFT.Ln)
            recipb = recp.tile([128, HALF], F32, tag="recipb", name=f"rec{h}")
            nc.scalar.activation(recipb, lnt, AFT.Exp, scale=-1.0)
            for ec in range(CCH):
                av = avp.tile([128, HALF], F32, tag="av", name=f"av{h}_{ec}")
                for t in range(NP):
                    for s in range(2):
                        ssl = slice(s * 512, (s + 1) * 512)
                        gsl = slice(h * HALF + s * 512, h * HALF + (s + 1) * 512)
                        nc.tensor.matmul(
                            av[:, ssl],
                            lhsT=vaug[t][:, :, ec * 128 : (ec + 1) * 128],
                            rhs=ex[t][:, :, gsl],
                            start=(t == 0),
                            stop=(t == NP - 1),
                            perf_mode=DR,
                        )
                fin = finp.tile([128, HALF], F32, tag="fin", name=f"fin{h}_{ec}")
                nc.vector.tensor_mul(fin, av, recipb)
                nc.vector.tensor_add(fin, fin, xg[ec][:, hsl])
                nc.sync.dma_start(out=out_d.ap()[ec, :, hsl], in_=fin)


_PROGRAM_CACHE = {}


def _get_program():
    if "nc" in _PROGRAM_CACHE:
        return _PROGRAM_CACHE["nc"]
    nc = bacc.Bacc("TRN2", target_bir_lowering=False, debug=False)
    x_d = nc.dram_tensor("x_loc", [CCH, 128, NLOC], F32, kind="ExternalInput")
    xb_d = nc.dram_tensor("x_bf", [CCH, 128, NLOC], BF16, kind="ExternalInput")
    y_d = nc.dram_tensor("y_full", [CCH, 128, N], BF16, kind="ExternalInput")
    wq_d = nc.dram_tensor("wq_t", [128, CCH, CQK], BF16, kind="ExternalInput")
    wk_d = nc.dram_tensor("wk_t", [128, CCH, CQK], BF16, kind="ExternalInput")
    wv_d = nc.dram_tensor("wv_t", [128, CCH, C], BF16, kind="ExternalInput")
    bq_d = nc.dram_tensor("bq_c", [CQK, 1], F32, kind="ExternalInput")
    bk_d = nc.dram_tensor("bk_c", [CQK, 1], F32, kind="ExternalInput")
    bv_d = nc.dram_tensor("bv2", [128, CCH], F32, kind="ExternalInput")
    g_d = nc.dram_tensor("gamma_b", [128, 1], F32, kind="ExternalInput")
    out_d = nc.dram_tensor("out_loc", [CCH, 128, NLOC], F32, kind="ExternalOutput")
    with tile.TileContext(nc) as tc, contextlib.ExitStack() as ctx:
        _trace_kernel(
            ctx, tc, x_d, xb_d, y_d, wq_d, wk_d, wv_d, bq_d, bk_d, bv_d, g_d, out_d
        )
    nc.compile()
    _PROGRAM_CACHE["nc"] = nc
    return nc


def _make_in_maps(inputs):
    import ml_dtypes

    BF = ml_dtypes.bfloat16
    x = np.ascontiguousarray(inputs["x"], dtype=np.float32).reshape(B, C, N)
    y = np.ascontiguousarray(
        np.asarray(inputs["y"], np.float32).astype(BF).reshape(B, C, N)
    )
    wq_t = np.ascontiguousarray(
        np.asarray(inputs["Wq"], np.float32)
        .astype(BF).T.reshape(CCH, 128, CQK).transpose(1, 0, 2)
    )
    wk_t = np.ascontiguousarray(
        np.asarray(inputs["Wk"], np.float32)
        .astype(BF).T.reshape(CCH, 128, CQK).transpose(1, 0, 2)
    )
    wv_t = np.ascontiguousarray(
        np.asarray(inputs["Wv"], np.float32)
        .astype(BF).T.reshape(CCH, 128, C).transpose(1, 0, 2)
    )
    bq_c = np.ascontiguousarray(np.asarray(inputs["bq"], np.float32).reshape(CQK, 1))
    bk_c = np.ascontiguousarray(np.asarray(inputs["bk"], np.float32).reshape(CQK, 1))
    bv2 = np.ascontiguousarray(np.asarray(inputs["bv"], np.float32).reshape(CCH, 128).T)
    gamma_b = np.full(
        (128, 1), float(np.asarray(inputs["gamma"]).reshape(-1)[0]), np.float32
    )

    in_maps = []
    for core in range(NCORES):
        b, h = divmod(core, 2)
        x_loc = np.ascontiguousarray(
            x[b, :, h * NLOC : (h + 1) * NLOC].reshape(CCH, 128, NLOC)
        )
        x_bf = np.ascontiguousarray(x_loc.astype(BF))
        y_full = np.ascontiguousarray(y[b].reshape(CCH, 128, N))
        in_maps.append(
            {
                "x_loc": x_loc,
                "x_bf": x_bf,
                "y_full": y_full,
                "wq_t": wq_t,
                "wk_t": wk_t,
                "wv_t": wv_t,
                "bq_c": bq_c,
                "bk_c": bk_c,
                "bv2": bv2,
                "gamma_b": gamma_b,
            }
        )
    return in_maps


def _assemble(results):
    out = np.empty((B, C, N), np.float32)
    for core in range(NCORES):
        b, h = divmod(core, 2)
        out[b, :, h * NLOC : (h + 1) * NLOC] = results[core]["out_loc"].reshape(
            C, NLOC
        )
    return out.reshape(B, C, 64, 64)


def run(inputs, trace=False, **kwargs):
    """Run the kernel; returns (full_output, BassKernelResults)."""
    nc = _get_program()
    in_maps = _make_in_maps(inputs)
    res = run_bass_kernel_spmd(
        nc, in_maps, core_ids=list(range(NCORES)), trace=trace, **kwargs
    )
    return _assemble(res.results), res


def kernel(**inputs) -> np.ndarray:
    out, _ = run(inputs, trace=False)
    return out


# revision 19
# speedup vs baseline: 1.1110x; 1.1110x over previous
"""Trainium2 Bass kernel for nn_CrossAttention (B=4, C=256, N=64*64=4096, CQK=32).

Reference computation:
    q = Wq @ xf + bq          [B, N, 32]
    k = Wk @ yf + bk          [B, 32, N]
    v = Wv @ yf + bv          [B, 256, N]
    attn = softmax(q @ k)     [B, N, N]
    out = gamma * (v @ attn^T) + x

Sharding: 8 cores = batch(4) x query-half(2). Each core owns 2048 query
positions of one sample and all 4096 keys of that sample.

v3 design notes (calibrated against measured traces):
  - biases are folded into the energy contraction via augmented projection
    rows (host-prepped): q_hat = [Wq x; (bk^T Wq) x; 1], k_hat = [Wk y; 1;
    (bq^T Wk) y]; the constant bq.bk term is softmax-invariant and dropped.
    No bias-add instructions at all.
  - energy: per key chunk, 4 matmuls [128, 512] sharing one stationary kT
    chunk into a [128, 2048] 4-bank PSUM tile, double-buffered.
  - exp split across TWO engines (the scalar ACTIVATE at 1 elem/cycle/lane
    was the wall): scalar does Exp for most chunks; the vector engine
    computes fp8 exp directly via the bit trick
       fp8e4_bits(e^x) ~= uint8(11.5416*x + 56.0)
    (one tensor_scalar psum->uint8, bitcast as fp8e4). The +-4% weight error
    is softmax-consistent and far inside the 2e-2 tolerance.
  - AV with v stationary, exp streamed fp8-DoubleRow (measured 1.0
    cyc/out-col warm): out arrives in final [e, n] orientation, no
    transposes. Softmax denominator via an all-(1/gamma) stationary
    (broadcast across partitions), reciprocal via Ln+Exp(-x) on the scalar
    engine, then normalize+residual as two vector ops per tile.
  - v projection accumulates pairs in a [128, 512] PSUM tile so one cast
    per PAIR produces the fp8 DoubleRow vaug tile; casts split between
    scalar (Copy) and vector to dodge the DVE drain penalty.
"""

import contextlib

import numpy as np

import concourse.mybir as mybir
import concourse.tile as tile
from concourse import bacc
from concourse.bass_utils import run_bass_kernel_spmd

F32 = mybir.dt.float32
F8 = mybir.dt.float8e4
U8 = mybir.dt.uint8
BF16 = mybir.dt.bfloat16
AFT = mybir.ActivationFunctionType
DR = mybir.MatmulPerfMode.DoubleRow
MUL = mybir.AluOpType.mult
ADD = mybir.AluOpType.add

B = 4
C = 256
CQK = 32
N = 4096  # 64 * 64
NCORES = 8
NLOC = N // 2  # 2048 queries per core
CCH = C // 128  # 2 channel chunks
MC = N // 128  # 32 key chunks
NP = MC // 2  # 16 key pairs (DoubleRow)
HALF = NLOC // 2  # 1024: AV accumulates per query-half (PSUM budget)
NPROJ = 64  # projected rows, padded to a partition-aligned count:
# q_hat: rows 0-31 Wq, 32 bk^T Wq, 33 ones (DMA), 34-63 zero
# k_hat: rows 0-31 Wk, 32 ones (DMA), 33 bq^T Wk, 34-63 zero
# fp8e4 bit-trick exp: bits = EXP_A * x + EXP_B, byte bitcast as fp8e4m3
EXP_A = 11.541560327111707  # 8 / ln(2)
EXP_B = 56.0  # 8 * fp8e4 exponent bias (7)
# chunks whose exp runs on the vector engine (bit trick); rest on scalar
DVE_CHUNKS = frozenset(c for c in range(6, 30) if c % 2 == 0)


def _trace_kernel(
    ctx, tc, x_d, xb_d, y_d, ones_d, wq_d, wk_d, wv_d, bv_d, g_d, out_d
):
    nc = tc.nc

    const = ctx.enter_context(tc.tile_pool(name="const", bufs=1))
    big = ctx.enter_context(tc.tile_pool(name="big", bufs=1))
    vaugp = ctx.enter_context(tc.tile_pool(name="vaugp", bufs=NP))
    expp = ctx.enter_context(tc.tile_pool(name="expp", bufs=NP))
    recp = ctx.enter_context(tc.tile_pool(name="recp", bufs=2))
    finp = ctx.enter_context(tc.tile_pool(name="finp", bufs=2))

    # ---- zero pads first (vector engine is idle at t=0) ----
    kT_sb = big.tile([128, N], BF16, tag="kT_sb")
    qT_sb = big.tile([128, NLOC], BF16, tag="qT_sb")
    nc.vector.memset(kT_sb[NPROJ:, :], 0.0)
    nc.vector.memset(qT_sb[NPROJ:, :], 0.0)

    # ---- constant / weight loads (weights pre-cast to bf16 on host) ----
    wq_b = const.tile([128, CCH, NPROJ], BF16, tag="wq_b")
    nc.sync.dma_start(out=wq_b, in_=wq_d.ap())
    wk_b = const.tile([128, CCH, NPROJ], BF16, tag="wk_b")
    nc.sync.dma_start(out=wk_b, in_=wk_d.ap())
    wv_b = const.tile([128, CCH, C], BF16, tag="wv_b")
    nc.sync.dma_start(out=wv_b, in_=wv_d.ap())
    bv_sb = const.tile([128, CCH], F32, tag="bv_sb")
    nc.sync.dma_start(out=bv_sb, in_=bv_d.ap())
    g_sb = const.tile([128, 1], F32, tag="g_sb")
    nc.sync.dma_start(out=g_sb, in_=g_d.ap())
    gbv_sb = const.tile([128, CCH], F32, tag="gbv_sb")
    nc.vector.tensor_scalar_mul(gbv_sb, bv_sb, g_sb)
    rg_sb = const.tile([128, 1], F32, tag="rg_sb")
    nc.vector.reciprocal(rg_sb, g_sb)
    # all-(1/gamma) stationary operand for the denominator matmuls
    ones_g = const.tile([128, 2, 128], F8, tag="ones_g")
    nc.vector.memset(ones_g, 1.0)
    nc.vector.tensor_scalar_mul(ones_g, ones_g, rg_sb)

    # ---- activations in: x_b on sync ring; y split across sync+gpsimd
    # rings in m-quarters; fp32 x (residual) late on gpsimd ----
    x_b = []
    for cc in range(CCH):
        x_bt = big.tile([128, NLOC], BF16, tag=f"x_b{cc}", name=f"x_b{cc}")
        nc.sync.dma_start(out=x_bt, in_=xb_d.ap()[cc])
        x_b.append(x_bt)
    y_b = [
        big.tile([128, N], BF16, tag=f"y_b{cc}", name=f"y_b{cc}")
        for cc in range(CCH)
    ]
    for q in range(4):
        sl = slice(q * 1024, (q + 1) * 1024)
        nc.gpsimd.dma_start(out=y_b[0][:, sl], in_=y_d.ap()[0, :, sl])
        nc.sync.dma_start(out=y_b[1][:, sl], in_=y_d.ap()[1, :, sl])
    xg = []
    for cc in range(CCH):
        x_t = big.tile([128, NLOC], F32, tag=f"xg{cc}", name=f"xg{cc}")
        for dd in range(2):
            sl = slice(dd * HALF, (dd + 1) * HALF)
            nc.gpsimd.dma_start(out=x_t[:, sl], in_=x_d.ap()[cc, :, sl])
        # fold gamma*bv into the residual on the (otherwise idle) gpsimd
        nc.gpsimd.tensor_scalar_add(x_t, x_t, gbv_sb[:, cc : cc + 1])
        xg.append(x_t)

    # ---- q/k projections (augmented rows, no bias ops) ----
    # q_hat rows 0..32 = [Wq; bk^T Wq] @ x, row 33 = ones (memset above)
    # k_hat rows 0..33 = [Wk; 0; bq^T Wk] @ y, row 32 overwritten to ones
    with contextlib.ExitStack() as pctx:
        ppq = pctx.enter_context(tc.tile_pool(name="ppq", bufs=1, space="PSUM"))
        pvp = pctx.enter_context(tc.tile_pool(name="pvp", bufs=4, space="PSUM"))
        pq = ppq.tile([NPROJ, NLOC], F32, tag="pp", name="pq")
        for s in range(4):
            ssl = slice(s * 512, (s + 1) * 512)
            for cc in range(CCH):
                nc.tensor.matmul(
                    pq[:, ssl],
                    lhsT=wq_b[:, cc, :],
                    rhs=x_b[cc][:, ssl],
                    start=(cc == 0),
                    stop=(cc == CCH - 1),
                )
        nc.vector.tensor_copy(qT_sb[0:NPROJ, :], pq)
        # q_hat ones row (33) over the zero col written by the copy
        nc.sync.dma_start(out=qT_sb[33:34, :], in_=ones_d.ap()[:, 0:NLOC])
        for nt in range(2):  # key halves of 2048
            msl = slice(nt * 2048, (nt + 1) * 2048)
            pk = ppq.tile([NPROJ, 2048], F32, tag="pp", name=f"pk{nt}")
            for s in range(4):
                ssl = slice(s * 512, (s + 1) * 512)
                gsl = slice(nt * 2048 + s * 512, nt * 2048 + (s + 1) * 512)
                for cc in range(CCH):
                    nc.tensor.matmul(
                        pk[:, ssl],
                        lhsT=wk_b[:, cc, :],
                        rhs=y_b[cc][:, gsl],
                        start=(cc == 0),
                        stop=(cc == CCH - 1),
                    )
            nc.vector.tensor_copy(kT_sb[0:NPROJ, msl], pk)
            # k_hat ones row (32) over the zero col written by the copy
            nc.sync.dma_start(out=kT_sb[32:33, msl], in_=ones_d.ap()[:, 0:2048])

        # ---- v projection -> fp8 DoubleRow pair tiles vaug[t][p, r, e] ----
        # one [128, 512] PSUM tile per pair (both chunks), one cast per pair;
        # casts alternate scalar Copy / vector copy
        vaug = []
        for t in range(NP):
            va = vaugp.tile([128, 2, C], F8, tag="vaug", name=f"vaug{t}")
            pv = pvp.tile([128, 2, C], F32, tag="pv", name=f"pv{t}")
            for r in range(2):
                mc = 2 * t + r
                for cc in range(CCH):
                    nc.tensor.matmul(
                        pv[:, r, :],
                        lhsT=y_b[cc][:, mc * 128 : (mc + 1) * 128],
                        rhs=wv_b[:, cc, :],
                        start=(cc == 0),
                        stop=(cc == CCH - 1),
                    )
            if t % 2 == 0:
                nc.scalar.activation(va, pv, AFT.Copy)
            else:
                nc.vector.tensor_copy(va, pv)
            vaug.append(va)

    # ---- energy + exp (two engines) ----
    ex = [
        expp.tile([128, 2, NLOC], F8, tag="exp", name=f"ex{t}") for t in range(NP)
    ]
    with contextlib.ExitStack() as pctx:
        pep = pctx.enter_context(tc.tile_pool(name="pep", bufs=2, space="PSUM"))
        for mc in range(MC):
            t, r = divmod(mc, 2)
            pe_t = pep.tile([128, NLOC], F32, tag="pe", name=f"pe{mc}")
            for s in range(4):
                ssl = slice(s * 512, (s + 1) * 512)
                nc.tensor.matmul(
                    pe_t[:, ssl],
                    lhsT=kT_sb[:, mc * 128 : (mc + 1) * 128],
                    rhs=qT_sb[:, ssl],
                    start=True,
                    stop=True,
                )
            if mc in DVE_CHUNKS:
                nc.vector.tensor_scalar(
                    out=ex[t][:, r, :].bitcast(U8),
                    in0=pe_t,
                    scalar1=EXP_A,
                    scalar2=EXP_B,
                    op0=MUL,
                    op1=ADD,
                )
            else:
                nc.scalar.activation(ex[t][:, r, :], pe_t, AFT.Exp)

    # ---- AV + denominator + normalize, per query half ----
    with contextlib.ExitStack() as pctx:
        dnp = pctx.enter_context(tc.tile_pool(name="dnp", bufs=1, space="PSUM"))
        avp = pctx.enter_context(tc.tile_pool(name="avp", bufs=3, space="PSUM"))
        for h in range(2):
            hsl = slice(h * HALF, (h + 1) * HALF)
            dn = dnp.tile([128, HALF], F32, tag="dn", name=f"dn{h}")
            for t in range(NP):
                for s in range(2):
                    ssl = slice(s * 512, (s + 1) * 512)
                    gsl = slice(h * HALF + s * 512, h * HALF + (s + 1) * 512)
                    nc.tensor.matmul(
                        dn[:, ssl],
                        lhsT=ones_g,
                        rhs=ex[t][:, :, gsl],
                        start=(t == 0),
                        stop=(t == NP - 1),
                        perf_mode=DR,
                    )
            # recipb = gamma / denom (dn = denom/gamma) via exp(-ln(x));
            # Ln and Exp share one activation table set
            lnt = recp.tile([128, HALF], F32, tag="lnt", name=f"lnt{h}")
            nc.scalar.activation(lnt, dn, AFT.Ln)
            recipb = recp.tile([128, HALF], F32, tag="recipb", name=f"rec{h}")
            nc.scalar.activation(recipb, lnt, AFT.Exp, scale=-1.0)
            for ec in range(CCH):
                av = avp.tile([128, HALF], F32, tag="av", name=f"av{h}_{ec}")
                for t in range(NP):
                    for s in range(2):
                        ssl = slice(s * 512, (s + 1) * 512)
                        gsl = slice(h * HALF + s * 512, h * HALF + (s + 1) * 512)
                        nc.tensor.matmul(
                            av[:, ssl],
                            lhsT=vaug[t][:, :, ec * 128 : (ec + 1) * 128],
                            rhs=ex[t][:, :, gsl],
                            start=(t == 0),
                            stop=(t == NP - 1),
                            perf_mode=DR,
                        )
                fin = finp.tile([128, HALF], F32, tag="fin", name=f"fin{h}_{ec}")
                nc.vector.tensor_mul(fin, av, recipb)
                nc.vector.tensor_add(fin, fin, xg[ec][:, hsl])
                nc.sync.dma_start(out=out_d.ap()[ec, :, hsl], in_=fin)


_PROGRAM_CACHE = {}


def _get_program():
    if "nc" in _PROGRAM_CACHE:
        return _PROGRAM_CACHE["nc"]
    nc = bacc.Bacc("TRN2", target_bir_lowering=False, debug=False)
    x_d = nc.dram_tensor("x_loc", [CCH, 128, NLOC], F32, kind="ExternalInput")
    xb_d = nc.dram_tensor("x_bf", [CCH, 128, NLOC], BF16, kind="ExternalInput")
    y_d = nc.dram_tensor("y_full", [CCH, 128, N], BF16, kind="ExternalInput")
    ones_d = nc.dram_tensor("ones_row", [1, 4096], BF16, kind="ExternalInput")
    wq_d = nc.dram_tensor("wq_t", [128, CCH, NPROJ], BF16, kind="ExternalInput")
    wk_d = nc.dram_tensor("wk_t", [128, CCH, NPROJ], BF16, kind="ExternalInput")
    wv_d = nc.dram_tensor("wv_t", [128, CCH, C], BF16, kind="ExternalInput")
    bv_d = nc.dram_tensor("bv2", [128, CCH], F32, kind="ExternalInput")
    g_d = nc.dram_tensor("gamma_b", [128, 1], F32, kind="ExternalInput")
    out_d = nc.dram_tensor("out_loc", [CCH, 128, NLOC], F32, kind="ExternalOutput")
    with tile.TileContext(nc) as tc, contextlib.ExitStack() as ctx:
        _trace_kernel(
            ctx, tc, x_d, xb_d, y_d, ones_d, wq_d, wk_d, wv_d, bv_d, g_d, out_d
        )
    nc.compile()
    _PROGRAM_CACHE["nc"] = nc
    return nc


def _make_in_maps(inputs):
    import ml_dtypes

    BF = ml_dtypes.bfloat16
    x = np.ascontiguousarray(inputs["x"], dtype=np.float32).reshape(B, C, N)
    y = np.ascontiguousarray(
        np.asarray(inputs["y"], np.float32).astype(BF).reshape(B, C, N)
    )
    Wq = np.asarray(inputs["Wq"], np.float32)
    Wk = np.asarray(inputs["Wk"], np.float32)
    bq = np.asarray(inputs["bq"], np.float32)
    bk = np.asarray(inputs["bk"], np.float32)
    # augmented projections: bias terms become contraction rows (padded to
    # 64 rows; ones rows are DMA'd separately over the zero columns)
    wq_aug = np.zeros((NPROJ, C), np.float32)
    wq_aug[0:CQK] = Wq
    wq_aug[32] = bk @ Wq
    wk_aug = np.zeros((NPROJ, C), np.float32)
    wk_aug[0:CQK] = Wk
    wk_aug[33] = bq @ Wk
    wq_t = np.ascontiguousarray(
        wq_aug.astype(BF).T.reshape(CCH, 128, NPROJ).transpose(1, 0, 2)
    )
    wk_t = np.ascontiguousarray(
        wk_aug.astype(BF).T.reshape(CCH, 128, NPROJ).transpose(1, 0, 2)
    )
    ones_row = np.ones((1, 4096), BF)
    wv_t = np.ascontiguousarray(
        np.asarray(inputs["Wv"], np.float32)
        .astype(BF).T.reshape(CCH, 128, C).transpose(1, 0, 2)
    )
    bv2 = np.ascontiguousarray(np.asarray(inputs["bv"], np.float32).reshape(CCH, 128).T)
    gamma_b = np.full(
        (128, 1), float(np.asarray(inputs["gamma"]).reshape(-1)[0]), np.float32
    )

    in_maps = []
    for core in range(NCORES):
        b, h = divmod(core, 2)
        x_loc = np.ascontiguousarray(
            x[b, :, h * NLOC : (h + 1) * NLOC].reshape(CCH, 128, NLOC)
        )
        x_bf = np.ascontiguousarray(x_loc.astype(BF))
        y_full = np.ascontiguousarray(y[b].reshape(CCH, 128, N))
        in_maps.append(
            {
                "x_loc": x_loc,
                "x_bf": x_bf,
                "y_full": y_full,
                "ones_row": ones_row,
                "wq_t": wq_t,
                "wk_t": wk_t,
                "wv_t": wv_t,
                "bv2": bv2,
                "gamma_b": gamma_b,
            }
        )
    return in_maps


def _assemble(results):
    out = np.empty((B, C, N), np.float32)
    for core in range(NCORES):
        b, h = divmod(core, 2)
        out[b, :, h * NLOC : (h + 1) * NLOC] = results[core]["out_loc"].reshape(
            C, NLOC
        )
    return out.reshape(B, C, 64, 64)


def run(inputs, trace=False, **kwargs):
    """Run the kernel; returns (full_output, BassKernelResults)."""
    nc = _get_program()
    in_maps = _make_in_maps(inputs)
    res = run_bass_kernel_spmd(
        nc, in_maps, core_ids=list(range(NCORES)), trace=trace, **kwargs
    )
    return _assemble(res.results), res


def kernel(**inputs) -> np.ndarray:
    out, _ = run(inputs, trace=False)
    return out
